# revision 35
# baseline (speedup 1.0000x reference)
"""Trainium2 Bass kernel for a pre-LN transformer block (B=2,S=2048,H=1024,NH=16,FFN=4096).

Sharding: 8 cores, 512 tokens/core (4 cores per batch element). K/V are
exchanged within each batch group via four fp8 4-rank AllGathers (K and V
each in two head-halves) so attention on early head-pairs overlaps the later
gathers and the fixed CC-engine startup latency.

Precision: the attention path (h1T, q/k/v, probs, ctx, out-projection) runs
in fp8-e4m3 with x16-scaled weights (scales folded into existing bias-adds /
activation scales) and DoubleRow dual-fp8 matmuls (two contraction tiles per
instruction); probs are stored as 8*exp(s) to sit in fp8's normal range (the
scale cancels in the softmax normalize). The residual stream, LayerNorm and
the FFN stay bf16/fp32 - fp8 there measurably costs ~1.5e-2 rel error because
the MLP output is not diluted by a large residual.

Other structure: consolidated multi-tile DMAs, bn_stats LayerNorm, PE
transposes, per-head-pair softmax tail with a fused two-row reciprocal and a
K=1 broadcast matmul into the spare PSUM rows of the ctx tile (deferred into
the next head-pair's score stream so the PE never stalls on it), and one
exp-pair per head-pair offloaded to the DVE via a one-instruction Schraudolph
bf16 exp (int16 bitcast).

Self-contained: hardcodes shapes; builds the Bass program once and runs it via
run_bass_kernel_spmd on cores 0-7.
"""

import sys

for _p in ("/root/.axon_site/_ro/trn_rl_repo", "/opt/trn_rl_repo"):
    if _p not in sys.path:
        sys.path.append(_p)

import numpy as np
import ml_dtypes

# If BASS_TRACE is set but the axon NTFF hook module is missing, the trace
# path would crash on import; pre-register a no-op hook shim so tracing
# degrades gracefully instead.
try:
    import antenv.axon_hooks  # noqa: F401
except ImportError:
    import types as _types
    _m = _types.ModuleType("antenv.axon_hooks")
    _m._hook = None
    _m.get_axon_ntff_profile_hook = lambda: _m._hook
    _m.set_axon_ntff_profile_hook = lambda h: setattr(_m, "_hook", h)
    sys.modules["antenv.axon_hooks"] = _m

import bass_rust
import concourse.bass as bass
import concourse.mybir as mybir
import concourse.tile as tile
from concourse.bass_utils import run_bass_kernel_spmd

BF16 = mybir.dt.bfloat16
F32 = mybir.dt.float32
F8 = mybir.dt.float8e4
LN8 = 2.0794415416798357  # exp scale: probs stored as 8*exp(s), cancels in normalize
AF = mybir.ActivationFunctionType
NPBF16 = np.dtype(ml_dtypes.bfloat16)
NPF8 = np.dtype(ml_dtypes.float8_e4m3fn)
DR = mybir.MatmulPerfMode.DoubleRow
# Schraudolph fast-exp on DVE: bf16(x) ~= bitcast_i16(round(SCH_A*s + SCH_B))
# approximates 8*exp(s) (the +3 octave folds the LN8 prob scale); the
# sawtooth error (~2%) is per-key noise that washes out in the softmax mix.
SCH_A = 184.6650085
SCH_B = 16634.0
DVE_EXP_PAIRS = (7,)        # kt pairs whose exp runs on DVE (last pair measured best)

B, S, H, NH, DH, FFN = 2, 2048, 1024, 16, 64, 4096
NC = 8                      # cores
T = 512                     # tokens per core
NT = T // 128               # token tiles per core (4)
GROUPS = [[0, 1, 2, 3], [4, 5, 6, 7]]
G = 4                       # cores per batch group
NKT = 16                    # key tiles per batch (4 ranks x 4)
NHP = NH // 2               # head pairs (8)
EPS = 1e-3
VW = DH + 2                 # 66: V cols + 2 ones cols (even for dual-fp8 ldweights)
NG = FFN // 128             # 32 ffn row-groups

USE_DMA_TRANSPOSE = False   # xbar DMA transpose serializes ~6us/tile; PE wins

# ---------------------------------------------------------------------------
# Workaround: this walrus build rejects >1 inline sync-wait per instruction.
# After Tile scheduling, move excess waits onto single-wait NoOp carriers
# inserted immediately before the over-limit instruction (same engine, same
# block, so per-engine program order and wait semantics are preserved).
# ---------------------------------------------------------------------------
def _split_multiwait(nc, limit=1):
    n_new = 0
    for f in nc.m.functions:
        for blk in f.blocks:
            insts = blk.instructions
            out = []
            for ins in insts:
                si = getattr(ins, "sync_info", None)
                waits = list(si.on_wait) if si is not None else []
                if len(waits) > limit:
                    for i, w in enumerate(waits[:-limit]):
                        nop = mybir.InstNoOp(
                            name=f"{ins.name}_w{i}",
                            sync_info=mybir.SyncInfo(on_wait=[w], on_update=[]),
                            bass_nofuse=True,
                            engine=ins.engine,
                        )
                        out.append(nop)
                        n_new += 1
                    ins.sync_info = mybir.SyncInfo(
                        on_wait=waits[-limit:], on_update=list(si.on_update)
                    )
                out.append(ins)
            if len(out) != len(insts):
                blk.instructions = out
    return n_new


def _emit(tc, nc, io):
    """Emit the per-core program. io: dict of DRAM APs."""
    from contextlib import ExitStack

    x_d = io["x"]
    out_d = io["out"]

    s_outer = ExitStack()

    constp = s_outer.enter_context(tc.tile_pool(name="constp", bufs=1))
    dramp = s_outer.enter_context(tc.tile_pool(name="dramp", bufs=1, space="DRAM"))

    # ---- phase-A bulk loads, one descriptor-dense DMA each (SP queue) ----
    persp = s_outer.enter_context(tc.tile_pool(name="persp", bufs=1))
    x_all = persp.tile([128, NT, H], F32, name="x_all")
    xv = x_d.rearrange("(t p) h -> p t h", p=128)
    nc.sync.dma_start(x_all[:, 0:2, :], xv[:, 0:2, :])
    nc.sync.dma_start(x_all[:, 2:4, :], xv[:, 2:4, :])

    sA = ExitStack()
    wpool = sA.enter_context(tc.tile_pool(name="wpool", bufs=3))
    wk_sb = wpool.tile([128, 8, H], F8, tag="w3", name="wk_sb")
    nc.sync.dma_start(wk_sb[:], io["wk"].rearrange("(f p) h -> p f h", p=128))

    # constants / biases
    ident = constp.tile([128, 128], BF16)
    nc.sync.dma_start(ident[:], io["ident"][:])
    ident_f8 = constp.tile([128, 128], F8)
    nc.sync.dma_start(ident_f8[:], io["ident_f8"][:])
    ones_row = constp.tile([1, 128], BF16)
    nc.sync.dma_start(ones_row[:], io["ones_row"][:])
    # bqki: cols 0:8 = bq, 8:16 = bk, 16:48 = bi (all [128, n])
    bqki = constp.tile([128, 48], F32)
    nc.sync.dma_start(bqki[:], io["bqki"][:])
    # bvpo: [1, 3H] bf16: bv | bproj | bo
    bvpo = constp.tile([1, 3 * H], BF16)
    nc.sync.dma_start(bvpo[:], io["bvpo"][:])
    eps_t = constp.tile([128, 1], F32)
    nc.gpsimd.memset(eps_t[:], float(EPS))
    ln8_t = constp.tile([128, 1], F32)
    nc.gpsimd.memset(ln8_t[:], float(LN8))

    wq_sb = wpool.tile([128, 8, H], F8, tag="w3", name="wq_sb")
    nc.sync.dma_start(wq_sb[:], io["wq"].rearrange("(f p) h -> p f h", p=128))
    wv_sb = wpool.tile([128, 8, H], F8, tag="w3", name="wv_sb")
    nc.sync.dma_start(wv_sb[:], io["wv"].rearrange("(f p) h -> p f h", p=128))

    # persistent activations
    x2_all = persp.tile([128, NT, H], F32, name="x2_all")
    qT_all = persp.tile([128, 8, T], F8, name="qT_all")
    ctxT_all = persp.tile([128, 8 * T], F8, name="ctxT_all")
    h2T_all = persp.tile([128, 8, T], BF16, name="h2T_all")
    wproj_sb = persp.tile([128, 8, H], F8, name="wproj_sb")

    # fp8 collective buffers: K and V, each split by head-half
    cc_k_in_a = dramp.tile([512, 512], F8)
    cc_k_in_b = dramp.tile([512, 512], F8)
    cc_k_out_a = dramp.tile([G * 512, 512], F8)
    cc_k_out_b = dramp.tile([G * 512, 512], F8)
    cc_v_in_a = dramp.tile([512, 512], F8)
    cc_v_in_b = dramp.tile([512, 512], F8)
    cc_v_out_a = dramp.tile([G * 512, 512], F8)
    cc_v_out_b = dramp.tile([G * 512, 512], F8)

    def layer_norm_stats(pool, x_slice):
        """x_slice [128,H] f32 -> (rs [128,1], nmr [128,1]) in SBUF."""
        stats = pool.tile([128, 2, 6], F32, tag="ln_st")
        mv = pool.tile([128, 2], F32, tag="ln_mv")
        std = pool.tile([128, 1], F32, tag="ln_std")
        rs = pool.tile([128, 1], F32, tag="ln_rs")
        nmr = pool.tile([128, 1], F32, tag="ln_nmr")
        xc = x_slice.rearrange("p (n c) -> p n c", c=512)
        nc.vector.bn_stats(out=stats[:, 0, :], in_=xc[:, 0, :])
        nc.vector.bn_stats(out=stats[:, 1, :], in_=xc[:, 1, :])
        nc.vector.bn_aggr(out=mv[:], in_=stats[:])
        nc.scalar.activation(std[:], mv[:, 1:2], AF.Sqrt, bias=eps_t[:])
        nc.vector.reciprocal(rs[:], std[:])
        nc.vector.tensor_mul(nmr[:], mv[:, 0:1], rs[:])
        nc.vector.tensor_scalar_mul(nmr[:], nmr[:], -1.0)
        return rs, nmr

    # =====================================================================
    # Phase A: load x, LN1 -> h1, h1T (xbar transpose), kT, v, qT.
    # K/V AllGathers (by head-half) dispatched as soon as inputs land.
    # =====================================================================
    lnp = sA.enter_context(tc.tile_pool(name="lnp", bufs=2))
    h1p = sA.enter_context(tc.tile_pool(name="h1p", bufs=2))
    h1Tp = sA.enter_context(tc.tile_pool(name="h1Tp", bufs=1))
    ktp = sA.enter_context(tc.tile_pool(name="ktp", bufs=1))
    vlp = sA.enter_context(tc.tile_pool(name="vlp", bufs=2))
    mmpsA = sA.enter_context(tc.tile_pool(name="mmpsA", bufs=3, space="PSUM"))
    tpsA = None
    stgA = None
    if not USE_DMA_TRANSPOSE:
        tpsA = sA.enter_context(tc.tile_pool(name="tpsA", bufs=2, space="PSUM"))
        stgA = sA.enter_context(tc.tile_pool(name="stgA", bufs=2))

    h1T_all = h1Tp.tile([128, 8, T], F8, name="h1T_all")
    ktA_sb = ktp.tile([128, 4, T], F8, name="ktA_sb")
    ktB_sb = ktp.tile([128, 4, T], F8, name="ktB_sb")

    def transpose_tile(dstT_all, h_tile, t):
        """h_tile [128, H] bf16 -> dstT_all[:, fb, t*128:(t+1)*128] for all fb."""
        dst = dstT_all.rearrange("p f (tt c) -> p f tt c", c=128)[:, :, t, :]
        if USE_DMA_TRANSPOSE:
            nc.scalar.dma_start_transpose(dst, h_tile)
        else:
            for fb in range(8):
                ps = tpsA.tile([128, 128], BF16, tag="tp")
                nc.tensor.transpose(ps[:], h_tile[:, fb * 128:(fb + 1) * 128], ident[:])
                nc.vector.tensor_copy(dst[:, fb, :], ps[:])

    for t in range(NT):
        rs, nmr = layer_norm_stats(lnp, x_all[:, t, :])
        h1 = h1p.tile([128, H], BF16, tag="h1")
        nc.scalar.activation(h1[:], x_all[:, t, :], AF.Identity, bias=nmr[:], scale=rs[:])
        transpose_tile(h1T_all, h1[:], t)

    def ag(cc_in, cc_out):
        nc.gpsimd.collective_compute(
            "AllGather", mybir.AluOpType.bypass, replica_groups=GROUPS,
            ins=[cc_in.opt()], outs=[cc_out.opt()])

    # kT feature-major: [128 feats(head pair), T local keys] per ct
    def emit_k_quarter(cts, dst):
        for ct in cts:
            ps = mmpsA.tile([128, T], F32, tag="mmA")
            for j in range(4):
                nc.tensor.matmul(
                    ps[:],
                    wk_sb[:, 2 * j:2 * j + 2, ct * 128:(ct + 1) * 128],
                    h1T_all[:, 2 * j:2 * j + 2, :],
                    start=(j == 0), stop=(j == 3), perf_mode=DR,
                )
            nc.vector.tensor_scalar(dst[:, ct % 4, :], ps[:], 1.0 / 16, bqki[:, 8 + ct:9 + ct],
                                    op0=mybir.AluOpType.mult, op1=mybir.AluOpType.add)

    # v token-major, feature half cc: [128 tok, 512]
    def emit_v_half(cc, vloc, cc_v_in):
        for t in range(NT):
            ps = mmpsA.tile([128, 512], F32, tag="mmA")
            for j in range(4):
                nc.tensor.matmul(
                    ps[:],
                    h1T_all[:, 2 * j:2 * j + 2, t * 128:(t + 1) * 128],
                    wv_sb[:, 2 * j:2 * j + 2, cc * 512:(cc + 1) * 512],
                    start=(j == 0), stop=False, perf_mode=DR,
                )
            nc.tensor.matmul(ps[:], ones_row[:], bvpo[:, cc * 512:(cc + 1) * 512],
                             start=False, stop=True)
            nc.scalar.activation(vloc[:, t, :], ps[:], AF.Copy, scale=1.0 / 16)
        nc.sync.dma_start(
            cc_v_in.rearrange("(t p) f -> p t f", p=128), vloc[:])

    def emit_q_quarter(cts):
        for ct in cts:
            ps = mmpsA.tile([128, T], F32, tag="mmA")
            for j in range(4):
                nc.tensor.matmul(
                    ps[:],
                    wq_sb[:, 2 * j:2 * j + 2, ct * 128:(ct + 1) * 128],
                    h1T_all[:, 2 * j:2 * j + 2, :],
                    start=(j == 0), stop=(j == 3), perf_mode=DR,
                )
            nc.vector.tensor_scalar(qT_all[:, ct, :], ps[:], 1.0 / 16, bqki[:, ct:ct + 1],
                                    op0=mybir.AluOpType.mult, op1=mybir.AluOpType.add)

    # interleave projections with the four fp8 gathers: K-A first (hp0
    # scores), then V-A (hp0 ctx), K-B, V-B. CC runs them back-to-back.
    emit_k_quarter(range(0, 4), ktA_sb)
    nc.sync.dma_start(
        cc_k_in_a.rearrange("(c p) k -> p c k", p=128), ktA_sb[:])
    ag(cc_k_in_a, cc_k_out_a)
    vloc_a = vlp.tile([128, NT, 512], F8, tag="vloc", name="vloc_a")
    emit_v_half(0, vloc_a, cc_v_in_a)
    ag(cc_v_in_a, cc_v_out_a)
    emit_k_quarter(range(4, 8), ktB_sb)
    nc.sync.dma_start(
        cc_k_in_b.rearrange("(c p) k -> p c k", p=128), ktB_sb[:])
    ag(cc_k_in_b, cc_k_out_b)
    emit_q_quarter(range(0, 4))
    vloc_b = vlp.tile([128, NT, 512], F8, tag="vloc", name="vloc_b")
    emit_v_half(1, vloc_b, cc_v_in_b)
    ag(cc_v_in_b, cc_v_out_b)
    emit_q_quarter(range(4, 8))

    nc.sync.dma_start(wproj_sb[:], io["wproj"].rearrange("(f p) h -> p f h", p=128))

    sA.close()

    # =====================================================================
    # Phase B: attention. scores^T per key-tile (row-packed head pairs),
    # exp on ACT, ctx^T via V'=[V|ones] (M=65), fast normalize at hp end.
    # =====================================================================
    sB = ExitStack()
    ktpool = sB.enter_context(tc.tile_pool(name="ktpool", bufs=5))
    vstg = sB.enter_context(tc.tile_pool(name="vstg", bufs=3))
    spool = sB.enter_context(tc.tile_pool(name="spool", bufs=2, space="PSUM"))
    cpool = sB.enter_context(tc.tile_pool(name="cpool", bufs=2, space="PSUM"))
    ppool = sB.enter_context(tc.tile_pool(name="ppool", bufs=6))
    ppool16 = sB.enter_context(tc.tile_pool(name="ppool16", bufs=3))
    rpool = sB.enter_context(tc.tile_pool(name="rpool", bufs=3))
    vsbp = sB.enter_context(tc.tile_pool(name="vsbp", bufs=1))
    vsb_a = vsbp.tile([128, NKT, 8, VW], F8, name="vsb_a")
    vsb_b = vsbp.tile([128, NKT, 8, VW], F8, name="vsb_b")

    # ones columns for all key tiles (DVE; no data deps)
    nc.vector.memset(vsb_a[:, :, :, DH:VW], 1.0)
    nc.vector.memset(vsb_b[:, :, :, DH:VW], 1.0)

    def load_kt(hp):
        """kT for head pair hp: [128 feats, 4 rank-blocks, 512 keys]."""
        kt = ktpool.tile([128, G, T], F8, tag="kt", name=f"kt{hp}")
        cko, hpo = (cc_k_out_a, hp) if hp < 4 else (cc_k_out_b, hp - 4)
        src = cko.rearrange("(g c p) k -> c p g k", g=G, c=4, p=128)[hpo]
        nc.sync.dma_start(kt[:], src)
        return kt

    def load_v_half(vsb, cc_v_out):
        """Interleave gathered V [tokens, 512] into vsb [128, kt, head, VW]."""
        for g in range(G):
            vplain = vstg.tile([128, NT, 512], F8, tag="vplain", name=f"vp{g}")
            src = cc_v_out.rearrange("(g t p) f -> g p t f", g=G, p=128)[g]
            nc.sync.dma_start(vplain[:], src)
            for t in range(NT):
                kt = g * 4 + t
                nc.gpsimd.tensor_copy(
                    vsb[:, kt, :, 0:DH],
                    vplain[:, t, :].rearrange("p (h d) -> p h d", d=DH),
                )

    def attend_hp(hp, kt_hp, vsb, pending_tail):
        """Returns a closure finishing this hp's softmax-normalize; the PE
        part of the previous hp's tail is emitted mid-loop (at kt==3) so
        the PE queue never stalls waiting on the DVE reciprocal chain."""
        hh = (hp % 4) * 2  # head-within-half base index
        cps0 = cpool.tile([128, T], F32, tag="ctx0")
        cps1 = cpool.tile([128, T], F32, tag="ctx1")

        def emit_ctx(pi, pb2, first, last):
            if pb2.dtype == F8:
                for h, cps in enumerate((cps0, cps1)):
                    nc.tensor.matmul(
                        cps[0:VW, :],
                        vsb[:, 2 * pi:2 * pi + 2, hh + h, :],
                        pb2[:, :, h, :],
                        start=first, stop=last, perf_mode=DR,
                    )
            else:
                pbb = pb2[:].bitcast(BF16)
                for j in range(2):
                    for h, cps in enumerate((cps0, cps1)):
                        nc.tensor.matmul(
                            cps[0:VW, :],
                            vsb[:, 2 * pi + j, hh + h, :],
                            pbb[:, j, h, :],
                            start=first and j == 0,
                            stop=last and j == 1,
                        )

        # software-pipelined over kt-PAIRS: scores+exp fill a [128,2,2,512]
        # pair tile; ctx runs as fp8 DoubleRow (two key tiles accumulated
        # per instruction), lagging 2 pairs to ride out V-gather latency.
        NPAIR = NKT // 2
        pend = []
        for pi in range(NPAIR):
            if pi == 6 and pending_tail is not None:
                pending_tail()
                pending_tail = None
            on_dve = pi in DVE_EXP_PAIRS
            if on_dve:
                pb2 = ppool16.tile([128, 2, 2, T], mybir.dt.int16, tag="pb16", name="pb16")
            else:
                pb2 = ppool.tile([128, 2, 2, T], F8, tag="pb", name="pb")
            for j in range(2):
                kt = 2 * pi + j
                g, jj = kt // 4, kt % 4
                ps = spool.tile([128, 1024], F32, tag="ps", name="ps")
                nc.tensor.matmul(
                    ps[:, 0:T],
                    kt_hp[0:64, g, jj * 128:(jj + 1) * 128],
                    qT_all[0:64, hp, :],
                    start=True, stop=True, tile_position=(0, 0),
                )
                nc.tensor.matmul(
                    ps[:, T:1024],
                    kt_hp[64:128, g, jj * 128:(jj + 1) * 128],
                    qT_all[64:128, hp, :],
                    start=True, stop=True, tile_position=(64, 0),
                )
                if j == 1 and len(pend) >= 2:
                    emit_ctx(*pend.pop(0))
                if on_dve:
                    nc.vector.tensor_scalar(
                        pb2[:, j, :, :], ps[:], SCH_A, SCH_B,
                        op0=mybir.AluOpType.mult, op1=mybir.AluOpType.add)
                else:
                    nc.scalar.activation(pb2[:, j, :, :], ps[:], AF.Exp, bias=ln8_t[:])
            pend.append((pi, pb2, pi == 0, pi == NPAIR - 1))
        while pend:
            emit_ctx(*pend.pop(0))

        # normalize: both heads' 1/sumexp in ONE fused DVE reciprocal (the
        # [*,512] op costs the same at 1 or 2 partitions); broadcast via a
        # K=1 matmul into the same tile's spare rows 64:128 (no extra PSUM
        # bank) deferred deep into the next hp's score stream.
        se = rpool.tile([33, T], F32, tag="se")
        rc = rpool.tile([33, T], F32, tag="rc")
        rcb0 = rpool.tile([1, T], BF16, tag="rcb0")
        rcb1 = rpool.tile([1, T], BF16, tag="rcb1")
        nc.vector.tensor_copy(se[0:1, :], cps0[DH:DH + 1, :])
        nc.vector.tensor_copy(se[32:33, :], cps1[DH:DH + 1, :])
        nc.vector.reciprocal(rc[:], se[:])
        nc.vector.tensor_scalar_mul(rcb0[:], rc[0:1, :], 16.0)
        nc.vector.tensor_scalar_mul(rcb1[:], rc[32:33, :], 16.0)
        rcbs = (rcb0, rcb1)

        def tail():
            for h, cps in enumerate((cps0, cps1)):
                rbs = rpool.tile([64, T], F32, tag=f"rbs{h}")
                nc.tensor.matmul(cps[64:128, :], ones_row[:, 0:64], rcbs[h][:],
                                 start=True, stop=True, tile_position=(0, 64))
                nc.vector.tensor_copy(rbs[:], cps[64:128, :])
                nc.vector.tensor_mul(
                    ctxT_all[h * 64:(h + 1) * 64, hp * T:(hp + 1) * T],
                    cps[0:DH, :], rbs[:])
        return tail

    # half A
    kt_tiles = {}
    kt_tiles[0] = load_kt(0)
    kt_tiles[1] = load_kt(1)
    load_v_half(vsb_a, cc_v_out_a)
    kt_tiles[2] = load_kt(2)
    kt_tiles[3] = load_kt(3)
    tail = None
    for hp in range(4):
        tail = attend_hp(hp, kt_tiles[hp], vsb_a, tail)
    # half B
    kt_tiles[4] = load_kt(4)
    load_v_half(vsb_b, cc_v_out_b)
    for hp in range(4, NHP):
        if hp + 1 < NHP:
            kt_tiles[hp + 1] = load_kt(hp + 1)
        tail = attend_hp(hp, kt_tiles[hp], vsb_b, tail)
    tail()  # last hp's normalize, right before proj consumes ctxT

    sB.close()

    # =====================================================================
    # Phase C: proj (token-major) + residual -> x2, LN2 -> h2T
    # =====================================================================
    sC = ExitStack()
    lnp2 = sC.enter_context(tc.tile_pool(name="lnp2", bufs=2))
    h2p = sC.enter_context(tc.tile_pool(name="h2p", bufs=2))
    mmpsC = sC.enter_context(tc.tile_pool(name="mmpsC", bufs=3, space="PSUM"))
    tpsC = None
    stgC = None
    if not USE_DMA_TRANSPOSE:
        tpsC = sC.enter_context(tc.tile_pool(name="tpsC", bufs=2, space="PSUM"))
        stgC = sC.enter_context(tc.tile_pool(name="stgC", bufs=2))

    def transpose_tile_C(h_tile, t):
        dst = h2T_all.rearrange("p f (tt c) -> p f tt c", c=128)[:, :, t, :]
        if USE_DMA_TRANSPOSE:
            nc.scalar.dma_start_transpose(dst, h_tile)
        else:
            for fb in range(8):
                pst = tpsC.tile([128, 128], BF16, tag="tp")
                nc.tensor.transpose(pst[:], h_tile[:, fb * 128:(fb + 1) * 128], ident[:])
                nc.vector.tensor_copy(dst[:, fb, :], pst[:])

    h2_prev = None
    for t in range(NT):
        ctxTv = ctxT_all.rearrange("p (hp tt) -> p hp tt", tt=T)
        for cc in range(2):
            ps = mmpsC.tile([128, 512], F32, tag="pj")
            for j in range(4):
                nc.tensor.matmul(
                    ps[:],
                    ctxTv[:, 2 * j:2 * j + 2, t * 128:(t + 1) * 128],
                    wproj_sb[:, 2 * j:2 * j + 2, cc * 512:(cc + 1) * 512],
                    start=(j == 0), stop=False, perf_mode=DR,
                )
            nc.tensor.matmul(ps[:], ones_row[:], bvpo[:, H + cc * 512:H + (cc + 1) * 512],
                             start=False, stop=True)
            nc.vector.scalar_tensor_tensor(
                out=x2_all[:, t, cc * 512:(cc + 1) * 512],
                in0=ps[:], scalar=1.0 / 256,
                in1=x_all[:, t, cc * 512:(cc + 1) * 512],
                op0=mybir.AluOpType.mult, op1=mybir.AluOpType.add)
        if h2_prev is not None:
            transpose_tile_C(h2_prev[0][:], h2_prev[1])
        rs, nmr = layer_norm_stats(lnp2, x2_all[:, t, :])
        h2 = h2p.tile([128, H], BF16, tag="h2")
        nc.scalar.activation(h2[:], x2_all[:, t, :], AF.Identity, bias=nmr[:], scale=rs[:])
        h2_prev = (h2, t)
    transpose_tile_C(h2_prev[0][:], h2_prev[1])

    sC.close()

    # =====================================================================
    # Phase D+E fused: per g: wi matmuls + gelu -> h3T[g], then wo matmuls
    # for output columns 0:512 accumulate into 4 persistent psums.
    # Second pass re-reads h3T for output columns 512:1024.
    # Weight streams ride the gpsimd SWDGE queue (batched loads).
    # =====================================================================
    sD = ExitStack()
    h3p = sD.enter_context(tc.tile_pool(name="h3p", bufs=1))
    wip = sD.enter_context(tc.tile_pool(name="wip", bufs=3))
    wop = sD.enter_context(tc.tile_pool(name="wop", bufs=2))
    mmpsD = sD.enter_context(tc.tile_pool(name="mmpsD", bufs=3, space="PSUM"))
    wops = sD.enter_context(tc.tile_pool(name="wops", bufs=1, space="PSUM"))
    outp = sD.enter_context(tc.tile_pool(name="outp", bufs=2))

    h3T_all = h3p.tile([128, NG, T], BF16, name="h3T_all")

    WIB = 4   # wi groups per DMA
    WOB = 8   # wo groups per DMA

    wi_tiles = [None] * (NG // WIB)
    wo_tiles = {}

    def load_wi(b):
        wi = wip.tile([128, WIB, 8, 128], BF16, tag="wi", name=f"wi{b}")
        src = io["wi"][b * WIB:(b + 1) * WIB].rearrange("g p f c -> p g f c")
        nc.gpsimd.dma_start(wi[:], src)
        return wi

    def load_wo(cc, b):
        wo = wop.tile([128, WOB, 512], BF16, tag="wo", name=f"wo{cc}_{b}")
        src = io["wo"][:, cc * 512:(cc + 1) * 512].rearrange(
            "(bb g p) f -> bb p g f", g=WOB, p=128)[b]
        nc.gpsimd.dma_start(wo[:], src)
        return wo

    # prefetch schedule on the gpsimd queue (wi bufs=3, wo bufs=2 gate it)
    wi_tiles[0] = load_wi(0)
    wi_tiles[1] = load_wi(1)
    wo_tiles[(0, 0)] = load_wo(0, 0)
    wi_tiles[2] = load_wi(2)
    wo_tiles[(0, 1)] = load_wo(0, 1)

    psE = [wops.tile([128, 512], F32, tag=f"wo_ps{t}", name=f"wo_ps{t}") for t in range(NT)]
    for g in range(NG):
        b, gi = g // WIB, g % WIB
        if b + 3 < len(wi_tiles) and wi_tiles[b + 3] is None and gi == 0:
            wi_tiles[b + 3] = load_wi(b + 3)
        wob = g // WOB
        if gi == 1 and b % 2 == 1 and (0, b // 2 + 2) not in wo_tiles and b // 2 + 2 < 4:
            wo_tiles[(0, b // 2 + 2)] = load_wo(0, b // 2 + 2)
        wi = wi_tiles[b]
        ps = mmpsD.tile([128, T], F32, tag="wi_ps", name="wi_ps")
        for fb in range(8):
            nc.tensor.matmul(
                ps[:], wi[:, gi, fb, :], h2T_all[:, fb, :],
                start=(fb == 0), stop=(fb == 7),
            )
        nc.scalar.activation(h3T_all[:, g, :], ps[:],
                             AF.Gelu_apprx_tanh, bias=bqki[:, 16 + g:17 + g])
        wo = wo_tiles[(0, wob)]
        for t in range(NT):
            nc.tensor.matmul(
                psE[t][:],
                h3T_all[:, g, t * 128:(t + 1) * 128],
                wo[:, g % WOB, :],
                start=(g == 0), stop=False,
            )
    ostage = outp.tile([128, NT, 512], F32, tag="ostage", name="ostage0")
    ov0 = out_d[:, 0:512].rearrange("(t p) f -> p t f", p=128)
    for t in range(NT):
        nc.tensor.matmul(psE[t][:], ones_row[:], bvpo[:, 2 * H:2 * H + 512],
                         start=False, stop=True)
        nc.vector.tensor_add(ostage[:, t, :], psE[t][:], x2_all[:, t, 0:512])
        nc.sync.dma_start(ov0[:, t, :], ostage[:, t, :])

    # second pass: output columns 512:1024
    wo_tiles[(1, 0)] = load_wo(1, 0)
    wo_tiles[(1, 1)] = load_wo(1, 1)
    psE2 = [wops.tile([128, 512], F32, tag=f"wo_ps{t}", name=f"wo2_ps{t}") for t in range(NT)]
    for g in range(NG):
        wob = g // WOB
        if g % WOB == 1 and (1, wob + 2) not in wo_tiles and wob + 2 < 4:
            wo_tiles[(1, wob + 2)] = load_wo(1, wob + 2)
        wo = wo_tiles[(1, wob)]
        for t in range(NT):
            nc.tensor.matmul(
                psE2[t][:],
                h3T_all[:, g, t * 128:(t + 1) * 128],
                wo[:, g % WOB, :],
                start=(g == 0), stop=False,
            )
    ostage2 = outp.tile([128, NT, 512], F32, tag="ostage", name="ostage1")
    ov1 = out_d[:, 512:1024].rearrange("(t p) f -> p t f", p=128)
    for t in range(NT):
        nc.tensor.matmul(psE2[t][:], ones_row[:], bvpo[:, 2 * H + 512:3 * H],
                         start=False, stop=True)
        nc.vector.tensor_add(ostage2[:, t, :], psE2[t][:], x2_all[:, t, 512:1024])
        nc.sync.dma_start(ov1[:, t, :], ostage2[:, t, :])

    sD.close()
    s_outer.close()


def _build_program():
    nc = bass.Bass("TRN2", target_bir_lowering=False, debug=False, num_devices=NC)
    io = {}
    io["x"] = nc.dram_tensor("x", [T, H], F32, kind="ExternalInput").ap()
    io["wq"] = nc.dram_tensor("wq", [H, H], F8, kind="ExternalInput").ap()
    io["wk"] = nc.dram_tensor("wk", [H, H], F8, kind="ExternalInput").ap()
    io["wv"] = nc.dram_tensor("wv", [H, H], F8, kind="ExternalInput").ap()
    io["wproj"] = nc.dram_tensor("wproj", [H, H], F8, kind="ExternalInput").ap()
    io["wi"] = nc.dram_tensor("wi", [NG, 128, 8, 128], BF16, kind="ExternalInput").ap()
    io["wo"] = nc.dram_tensor("wo", [FFN, H], BF16, kind="ExternalInput").ap()
    io["bqki"] = nc.dram_tensor("bqki", [128, 48], F32, kind="ExternalInput").ap()
    io["bvpo"] = nc.dram_tensor("bvpo", [1, 3 * H], BF16, kind="ExternalInput").ap()
    io["ident"] = nc.dram_tensor("ident", [128, 128], BF16, kind="ExternalInput").ap()
    io["ident_f8"] = nc.dram_tensor("ident_f8", [128, 128], F8, kind="ExternalInput").ap()
    io["ones_row"] = nc.dram_tensor("ones_row", [1, 128], BF16, kind="ExternalInput").ap()
    io["out"] = nc.dram_tensor("out", [T, H], F32, kind="ExternalOutput").ap()
    with tile.TileContext(nc) as tc:
        _emit(tc, nc, io)
    _split_multiwait(nc)
    return nc


_PROGRAM = None
LAST_RESULTS = None


def kernel(x, ln1_scale, ln1_bias, qkv_w, qkv_b, proj_w, proj_b,
           ln2_scale, ln2_bias, wi_w, wi_b, wo_w, wo_b):
    global _PROGRAM, LAST_RESULTS
    x = np.asarray(x, np.float32)
    ln1_scale = np.asarray(ln1_scale, np.float32); ln1_bias = np.asarray(ln1_bias, np.float32)
    qkv_w = np.asarray(qkv_w, np.float32); qkv_b = np.asarray(qkv_b, np.float32)
    proj_w = np.asarray(proj_w, np.float32); proj_b = np.asarray(proj_b, np.float32)
    ln2_scale = np.asarray(ln2_scale, np.float32); ln2_bias = np.asarray(ln2_bias, np.float32)
    wi_w = np.asarray(wi_w, np.float32); wi_b = np.asarray(wi_b, np.float32)
    wo_w = np.asarray(wo_w, np.float32); wo_b = np.asarray(wo_b, np.float32)

    # fold LN affine params into the next matmul's weights/biases
    qkv_w_eff = ln1_scale[:, None] * qkv_w
    qkv_b_eff = qkv_b + ln1_bias @ qkv_w
    w3 = qkv_w_eff.reshape(H, NH, 3, DH)
    b3 = qkv_b_eff.reshape(NH, 3, DH)
    scale = 1.0 / np.sqrt(np.float32(DH))
    wq = (w3[:, :, 0, :] * scale).reshape(H, H)
    wk = w3[:, :, 1, :].reshape(H, H)
    wv = w3[:, :, 2, :].reshape(H, H)
    bq_v = (b3[:, 0, :] * scale).reshape(H)
    bk_v = b3[:, 1, :].reshape(H)
    bv_v = b3[:, 2, :].reshape(H)
    wi_eff = ln2_scale[:, None] * wi_w
    bi_v = wi_b + ln2_bias @ wi_w

    bqki = np.concatenate([
        bq_v.reshape(8, 128).T, bk_v.reshape(8, 128).T,
        bi_v.reshape(32, 128).T], axis=1).astype(np.float32)
    bvpo = np.concatenate([bv_v * 16, proj_b * 256, wo_b]).reshape(1, 3 * H)

    common = {
        "wq": (wq * 16).astype(NPF8), "wk": (wk * 16).astype(NPF8),
        "wv": (wv * 16).astype(NPF8),
        "wproj": (proj_w * 16).astype(NPF8),
        "wi": np.ascontiguousarray(
            wi_eff.astype(NPBF16).reshape(8, 128, 32, 128).transpose(2, 1, 0, 3)),
        "wo": wo_w.astype(NPBF16),
        "bqki": np.ascontiguousarray(bqki),
        "bvpo": bvpo.astype(NPBF16),
        "ident": np.eye(128, dtype=NPBF16),
        "ident_f8": np.eye(128, dtype=NPF8),
        "ones_row": np.ones((1, 128), NPBF16),
    }
    x_flat = x.reshape(B * S, H)
    in_maps = []
    for c in range(NC):
        m = dict(common)
        m["x"] = np.ascontiguousarray(x_flat[c * T:(c + 1) * T, :])
        in_maps.append(m)

    if _PROGRAM is None:
        _PROGRAM = _build_program()
    r = run_bass_kernel_spmd(_PROGRAM, in_maps, list(range(NC)))
    LAST_RESULTS = r
    out = np.concatenate([r.results[c]["out"] for c in range(NC)], axis=0)
    return out.reshape(B, S, H).astype(np.float32)
